# revision 21
# baseline (speedup 1.0000x reference)
"""Trainium2 Bass kernel for nn_Agent_57732950393167 (ragged_sequence).

Strategy
--------
Data-parallel over batches: 32 batches / 8 cores = 4 batches ("groups" g)
per core, each with V=8 vehicles -> 32 vehicles/core.

Key restructurings (vs. the reference):
 * nde = ndf @ W_ns ([T,N,384], 402MB) is NEVER materialized.  It is rank-8
   in the feature dim, so its three uses are folded:
     - K-part: compat_dyn[t,h,n] = sum_f ndf[t,n,f] * qw[t][f,h],
       qw = (Q/4) . W_nsK head-blocks  (an [8,8] matrix per vehicle)
     - V-part: heads_dyn = (sum_n attn*ndf) @ W_nsV  (attn-weighted feature
       sums AF[t,h,f], an [8,8] per vehicle)
     - L-part: logits_dyn[t,n] = sum_f ndf[t,n,f] * (W_nsL . final_Q[t])
 * Big matmuls batch the 64 (vehicle,head) rows of a batch-pair into one
   PE pass using block-diagonal stationary matrices (built on device via a
   replication matmul + select-mask; PE partition bases stay 32-aligned).
 * Every DMA ships in its exact SBUF destination layout (host prep), so
   each transfer is per-partition contiguous at line rate; rings are
   split sync=kt/wpack/mask, scalar=cpack/ndftm, gpsimd=rhsha/lt.
 * Program order: prime (PE clock warm-up riding the DMA window, tail
   gated on the first real data arrivals) -> phase A smalls -> ALL
   compat+softmax -> all transposes -> heads -> batched logits ->
   epilogue, so the PE stream is dense once data lands.
 * Softmax runs unnormalized (no max shift -- |scores| < ~15); the 1/sum
   is folded into the single heads-PSUM rescale via a [128,1] per-pair
   reciprocal vector.
 * Logits phase is fully batched: all 4 groups' logits accumulate into
   one [32, N] PSUM tile via zero-padded stationaries (group g's columns
   at 8g..8g+8), then ONE tanh evaluated as exp+reciprocal
   (tanh(y) = 1 - 2/(e^(2y)+1)) so the whole kernel uses a single ACT
   function-table set (ln/exp/identity/copy) -- no mid-kernel reloads.
 * Epilogue: one [32,N] max_with_indices / one Exp; lp = M - ln(S),
   prob = 1/(S*e^-M) via DVE reciprocal.

log(mask) is approximated by MASK_BIG*(mask-1) with MASK_BIG=50 (exact to
~1e-9 relative in the final softmax sums).
"""

import numpy as np

B, N, D, H, V = 32, 1024, 128, 8, 8
KS = D // H            # 16
F_V = 4
F_ND = 8
TANH_CLIP = 10.0
MASK_BIG = 50.0
NCORES = 8
G = B // NCORES        # 4 groups (batches) per core
NPAIR = G // 2         # 2 batch-pairs per core

_PROGRAM_CACHE = {}

CP_W = 849             # const pack width


def _build_const_pack():
    """[128, CP_W] f32: all device constants, one DMA."""
    KSl = KS
    cp = np.zeros((128, CP_W), dtype=np.float32)
    cp[:, 0:128] = np.eye(128, dtype=np.float32)                  # ident
    cp[:, 128:192] = np.tile(np.eye(64, dtype=np.float32), (2, 1))  # identpad
    cp[0:F_ND, 192:320] = np.tile(np.eye(F_ND, dtype=np.float32),
                                  (1, 16))                        # repl
    dm = np.zeros((128, 128), dtype=np.float32)                   # diagmask
    for p in range(128):
        g2, hv = divmod(p, 64)
        a = g2 * 8 + (hv % 8)
        dm[p, a * 8:(a + 1) * 8] = 1.0
    cp[:, 320:448] = dm
    bd = np.zeros((128, 128), dtype=np.float32)                   # bdsel
    for p in range(128):
        a = p // 8
        g2, v = divmod(a, 8)
        bd[p, g2 * 64 + v:g2 * 64 + 64:8] = 1.0
    cp[:, 448:576] = bd
    fw = np.zeros((128, 16), dtype=np.float32)                    # fwsel
    for p in range(128):
        fw[p, p // 8] = 1.0
    cp[:, 576:592] = fw
    hs = np.zeros((128, 64), dtype=np.float32)                    # hsel
    for hk in range(128):
        h = hk // KSl
        hs[hk, h * 8:(h + 1) * 8] = 1.0
    cp[:, 592:656] = hs
    hb = np.zeros((128, 128), dtype=np.float32)                   # hselb
    for d in range(128):
        h = d // KSl
        for g2 in range(2):
            hb[d, g2 * 64 + h * V:g2 * 64 + (h + 1) * V] = 1.0
    cp[:, 656:784] = hb
    rb = np.zeros((V, H * V), dtype=np.float32)                   # replbig
    for v in range(V):
        rb[v, v::V] = 1.0
    cp[0:V, 784:848] = rb
    cp[0:32, 848] = 8192.0 - 1024.0 * (np.arange(32) % 8)         # c8k32
    return cp


# --------------------------------------------------------------------------
# Device program
# --------------------------------------------------------------------------

def _build_program():
    import contextlib

    import concourse.bacc as bacc
    import concourse.tile as tile
    import concourse.mybir as mybir

    dt = mybir.dt
    f32 = dt.float32
    f32r = dt.float32r
    AF_EXP = mybir.ActivationFunctionType.Exp
    AF_LN = mybir.ActivationFunctionType.Ln
    AF_TANH = mybir.ActivationFunctionType.Tanh
    AF_IDENT = mybir.ActivationFunctionType.Identity
    AF_COPY = mybir.ActivationFunctionType.Copy
    OP = mybir.AluOpType
    AX = mybir.AxisListType

    nc = bacc.Bacc("TRN2", target_bir_lowering=False, debug=False,
                   num_devices=NCORES)

    # ---- external inputs (per-core shards, host-prepped layouts) ----
    kt_in = nc.dram_tensor("kt_in", [G, 128, N], f32r, kind="ExternalInput")
    lt_in = nc.dram_tensor("lt_in", [128, G * N], f32r,
                           kind="ExternalInput")
    rhsha_in = nc.dram_tensor("rhsha_in", [NPAIR, 128, 2 * N], f32r,
                              kind="ExternalInput")
    ndftm_in = nc.dram_tensor("ndftm_in", [G, 72, N], f32r,
                              kind="ExternalInput")
    mbs32_in = nc.dram_tensor("mbs32_in", [32, N], f32r,
                              kind="ExternalInput")
    wpack_in = nc.dram_tensor("wpack_in", [128, 564], f32,
                              kind="ExternalInput")

    res_out = nc.dram_tensor("res_out", [G, 4], f32, kind="ExternalOutput")

    cpack_c = nc.inline_tensor(_build_const_pack(), name="cpack_c")

    with tile.TileContext(nc) as tc:
        with contextlib.ExitStack() as ctx:
            sb = ctx.enter_context(tc.tile_pool(name="sb", bufs=1))
            scr = ctx.enter_context(tc.tile_pool(name="scr", bufs=4))
            acc = ctx.enter_context(
                tc.tile_pool(name="acc", bufs=2, space="PSUM"))
            tp = ctx.enter_context(
                tc.tile_pool(name="tp", bufs=3, space="PSUM"))
            flp = ctx.enter_context(
                tc.tile_pool(name="flp", bufs=1, space="PSUM"))

            def P(name, shape, dtype=f32):
                return sb.tile(shape, dtype, name=name, tag=name)

            def S(name, shape, dtype=f32):
                if shape[-1] >= 512:
                    return scr.tile(shape, dtype, name=name, tag="sbig",
                                    bufs=4)
                return scr.tile(shape, dtype, name=name, tag="ssml", bufs=8)

            # ================= persistent SBUF tiles =================
            kt = P("kt", [128, G * N], f32r)       # K^T  (g,n) cols
            lt = P("lt", [128, G * N], f32r)       # logitK^T
            ndftm = [P(f"ndftm{g}", [72, N], f32r) for g in range(G)]
            rhsha = [P(f"rhsha{p}", [128, 3 * N], f32r) for p in range(NPAIR)]
            attnt = [P(f"attntp{p}", [128, N]) for p in range(NPAIR)]
            attnnt = [P(f"attnnt{p}", [128, 8 * 128], f32r)
                      for p in range(NPAIR)]
            cpack = P("cpack", [128, CP_W])
            wpack = P("wpack", [128, 564])
            wpackr = P("wpackr", [128, 16], f32r)
            replbig_r = P("replbig_r", [V, H * V], f32r)
            mbs32 = P("mbs32", [32, N], f32r)      # 10 + 50*(mask-1)
            fctq = P("fctq", [128, G])
            queryt = P("queryt", [128, G * V])     # 0.25-scaled query^T
            blockq = [P(f"blockq{p}", [128, 128], f32r) for p in range(NPAIR)]
            bdq72 = [P(f"bdq72_{g}", [72, 64], f32r) for g in range(G)]
            ha_sb = [P(f"hasb{p}", [128, 384]) for p in range(NPAIR)]
            afdt = [P(f"afdt{p}", [F_ND, 128]) for p in range(NPAIR)]
            hct = [P(f"hctp{p}", [128, 2 * V]) for p in range(NPAIR)]
            fqt = [P(f"fqt{p}", [128, 2 * V], f32r) for p in range(NPAIR)]
            fq32g = [P(f"fq32g{g}", [128, 32], f32r) for g in range(G)]
            bdfw32 = [P(f"bdfw32_{g}", [64, 32], f32r) for g in range(G)]
            lgf32 = P("lgf32", [32, N])
            u32 = P("u32", [32, N])
            rinv_p = [P(f"rinvp{p}", [128, 1]) for p in range(NPAIR)]
            prime_sb = P("prime_sb", [128, 512])

            # ================= DMA issues =================
            # Everything rides ONE HWDGE ring (sync) in exact consumption
            # order: a single queue gets the full per-NC HBM bandwidth and
            # drains strictly FIFO, so arrival order == this issue order.
            # (Queues round-robin per packet, so splitting rings would give
            # bandwidth by packet size, not priority; and the tile
            # scheduler reorders dep-free instructions, so ordering must be
            # expressed by issue sequence on one engine.)
            nc.sync.dma_start(wpack[:], wpack_in.ap())
            nc.sync.dma_start(cpack[:], cpack_c.ap())
            for g in range(G):
                nc.sync.dma_start(kt[:, g * N:(g + 1) * N], kt_in[g])
                nc.sync.dma_start(ndftm[g][:], ndftm_in[g])
            nc.sync.dma_start(rhsha[0][:, 0:2 * N], rhsha_in[0])
            nc.sync.dma_start(rhsha[1][:, 0:2 * N], rhsha_in[1])
            nc.sync.dma_start(lt[:, 0:2 * N], lt_in.ap()[:, 0:2 * N])
            nc.sync.dma_start(lt[:, 2 * N:4 * N],
                              lt_in.ap()[:, 2 * N:4 * N])
            nc.sync.dma_start(mbs32[:], mbs32_in.ap())

            # ================= small setup on DVE ==================
            nc.vector.memset(prime_sb[:], 0.0)
            for g in range(G):
                # f32r tiles cannot be memset directly; copy zeros instead
                nc.vector.tensor_copy(fq32g[g][:], prime_sb[:, 0:32])
                nc.vector.tensor_copy(bdfw32[g][:], prime_sb[0:64, 0:32])

            # PE warm-up: back-to-back matmuls so the HAM un-throttles the
            # PE clock right as real data lands; the tail ones read the
            # first real arrivals so the chain stretches to meet them.
            for i in range(4):
                prime_ps = tp.tile([128, 512], f32, name=f"prime{i}",
                                   tag="tp")
                nc.tensor.matmul(prime_ps[:], prime_sb[:, 0:128],
                                 prime_sb[:], start=True, stop=True,
                                 skip_group_check=True)
            prime_k = tp.tile([128, 512], f32, name="prime_k", tag="tp")
            nc.tensor.matmul(prime_k[:], kt[:, 0:128], kt[:, 0:512],
                             start=True, stop=True, skip_group_check=True)
            prime_n = tp.tile([72, 512], f32, name="prime_n", tag="tp")
            nc.tensor.matmul(prime_n[:], ndftm[0][:, 0:72],
                             ndftm[0][:, 0:512],
                             start=True, stop=True, skip_group_check=True)

            # HAM keep-warm fillers: junk matmuls into a dedicated PSUM
            # bank, inserted at phase boundaries so the PE activity
            # monitor never sees an idle window and holds the 2.4 GHz
            # clock.  f32r with a 256-wide moving operand = 107 ns each
            # warm; the kt stationary is reused so LDW is elided.
            fl_ps = flp.tile([128, 256], f32, name="fl_ps", tag="fl")

            def filler(n, early=False):
                for _ in range(n):
                    if early:
                        nc.tensor.matmul(fl_ps[:, 0:128],
                                         prime_sb[:, 0:128],
                                         prime_sb[:, 0:128],
                                         start=True, stop=True,
                                         skip_group_check=True)
                    else:
                        nc.tensor.matmul(fl_ps[:], kt[:, 0:128],
                                         kt[:, 0:256],
                                         start=True, stop=True,
                                         skip_group_check=True)

            # const/weight slices
            ident = cpack[:, 0:128]
            identpad = cpack[:, 128:192]
            repl = cpack[0:F_ND, 192:320]
            diagmask = cpack[:, 320:448]
            bdsel = cpack[:, 448:576]
            fwsel = cpack[:, 576:592]
            hsel = cpack[:, 592:656]
            hselb = cpack[:, 656:784]
            replbig = cpack[0:V, 784:848]
            c8k32 = cpack[0:32, 848:849]
            wcs_hi = wpack[:, 0:128]
            wout = wpack[:, 128:256]
            wnsv = wpack[0:F_ND, 256:384]
            wcs_lo = wpack[0:F_V, 384:512]
            fct = wpack[:, 512:516]
            vdft = wpack[0:F_V, 516:548]
            nc.vector.tensor_copy(wpackr[:], wpack[:, 548:564])
            identpad_r = P("identpad_r", [64, 64], f32r)
            nc.vector.tensor_copy(identpad_r[:], cpack[0:64, 128:192])
            wnskt = wpackr[:, 0:F_ND]
            wnslt = wpackr[:, F_ND:2 * F_ND]

            # fctq = 0.25*fc^T
            nc.vector.tensor_scalar_mul(fctq[:], fct, 0.25)
            nc.vector.tensor_copy(replbig_r[:], replbig)

            # ================= phase A: query / qw smalls =================
            fct8a = S("fct8a", [128, G * V])
            for g in range(G):
                nc.scalar.activation(fct8a[:, g * V:(g + 1) * V],
                                     ident[:, 0:V], AF_IDENT,
                                     bias=fct[:, g:g + 1], scale=0.0)
            qt_ps = tp.tile([128, G * V], f32, name="qt_ps", tag="tp")
            nc.tensor.matmul(qt_ps[:], wcs_hi, fct8a[:],
                             start=True, stop=False, skip_group_check=True)
            nc.tensor.matmul(qt_ps[:], wcs_lo, vdft,
                             start=False, stop=True, skip_group_check=True)
            filler(3, early=True)
            for g in range(G):
                # queryt = 0.25*(cur + fc)
                nc.scalar.activation(queryt[:, g * V:(g + 1) * V],
                                     qt_ps[:, g * V:(g + 1) * V],
                                     AF_IDENT, bias=fctq[:, g:g + 1],
                                     scale=0.25)

            for p in range(NPAIR):
                # blockq[d, (g2,h,v)] = queryt[d, (g,v)] * (h == d//16)
                qview = (queryt[:, 2 * p * V:(2 * p + 2) * V]
                         .rearrange("d (g2 v) -> d g2 v", g2=2)
                         .unsqueeze(2).broadcast_to([128, 2, H, V]))
                nc.vector.tensor_tensor(
                    blockq[p].rearrange("d (g2 h v) -> d g2 h v", g2=2, h=H),
                    qview, hselb.rearrange("d (g2 h v) -> d g2 h v",
                                           g2=2, h=H),
                    OP.mult)
                # qw_all[f, (g2,h,v)] then replicate+mask into block-diag
                qw_ps = tp.tile([F_ND, 128], f32, name=f"qw_ps{p}", tag="tp")
                nc.tensor.matmul(qw_ps[:], wnskt, blockq[p][:],
                                 start=True, stop=True)
                qw_sbt = S(f"qw_sbt{p}", [F_ND, 128])
                nc.vector.tensor_copy(qw_sbt[:], qw_ps[:])
                qwr_ps = tp.tile([128, 128], f32, name=f"qwr_ps{p}", tag="tp")
                nc.tensor.matmul(qwr_ps[:], repl, qw_sbt[:],
                                 start=True, stop=True)
                filler(2, early=True)
                for g2 in range(2):
                    g = 2 * p + g2
                    gsl = slice(g2 * 64, (g2 + 1) * 64)
                    nc.vector.tensor_tensor(bdq72[g][0:64, :],
                                            qwr_ps[gsl, gsl],
                                            bdsel[gsl, gsl], OP.mult)
                    nc.vector.tensor_copy(bdq72[g][64:72, :], replbig)

            # ===== phases C/T/H/sm as functions, issued in consumption
            # order so the PE stream matches the single-ring DMA arrival
            # order: C0, C1, T0, H0, C2, C3, sm0, T1, H1, sm1, E.
            def phase_C(g):
                p, g2 = divmod(g, 2)
                gsl = slice(g2 * 64, (g2 + 1) * 64)
                compat = acc.tile([64, N], f32, name=f"compat{g}",
                                  tag="acc")
                # dyn + mask in one 72-row contraction; static after
                for half in range(2):
                    sl = slice(half * 512, (half + 1) * 512)
                    nc.tensor.matmul(
                        compat[:, sl], bdq72[g][:], ndftm[g][:, sl],
                        start=True, stop=False, skip_group_check=True)
                for half in range(2):
                    sl = slice(half * 512, (half + 1) * 512)
                    nc.tensor.matmul(
                        compat[:, sl], blockq[p][:, gsl],
                        kt[:, g * N:(g + 1) * N][:, sl],
                        start=False, stop=True, skip_group_check=True)
                # unnormalized exp into the pair tile (|compat| < ~15)
                rsum = S(f"rsum{g}", [64, 1])
                nc.scalar.activation(attnt[p][gsl, :], compat[:],
                                     AF_EXP, accum_out=rsum[:])
                nc.vector.reciprocal(rinv_p[p][gsl, :], rsum[:])

            def phase_T(p):
                # attn^T -> attn_n: [128,128] transposes, paired copies
                for c2 in range(4):
                    at_ps = tp.tile([128, 256], f32,
                                    name=f"at_ps{p}_{c2}", tag="tp")
                    for j in range(2):
                        c = 2 * c2 + j
                        nc.tensor.matmul(
                            at_ps[:, j * 128:(j + 1) * 128],
                            attnt[p][:, c * 128:(c + 1) * 128],
                            ident,
                            is_transpose=True,
                            start=True, stop=True,
                            skip_group_check=True)
                    dst = (attnnt[p]
                           .rearrange("q (c w) -> q c w", w=128)
                           [:, 2 * c2:2 * c2 + 2, :])
                    src_ap = at_ps.rearrange("q (j w) -> q j w", j=2)
                    if c2 % 2 == 0:
                        nc.scalar.activation(dst, src_ap, AF_COPY)
                    else:
                        nc.vector.tensor_copy(dst, src_ap)
                    filler(1)

            def phase_H(p):
                # heads+AF over the 3 contiguous rhsha regions (2-dim
                # free AP); normalization via rinv in the PSUM->SBUF move
                ha_ps = tp.tile([128, 384], f32, name=f"ha_ps{p}", tag="tp")
                rh = rhsha[p].rearrange("q (r w) -> q r w", r=3)
                for c in range(8):
                    nc.tensor.matmul(ha_ps[:],
                                     attnnt[p][:, c * 128:(c + 1) * 128],
                                     rh[:, :, c * 128:(c + 1) * 128],
                                     start=(c == 0), stop=(c == 7))
                nc.vector.tensor_scalar_mul(ha_sb[p][:], ha_ps[:],
                                            rinv_p[p][:])

            def phase_sm(p):
                # AF diag-extract -> AFd [128, F] -> AFd^T
                aftmp = S(f"aftmp{p}", [128, 128])
                nc.vector.tensor_tensor(aftmp[:], ha_sb[p][:, 256:384],
                                        diagmask, OP.mult)
                afd = S(f"afd{p}", [128, F_ND])
                nc.vector.tensor_reduce(
                    afd[:], aftmp.rearrange("q (a f) -> q f a", f=F_ND),
                    AX.X, OP.add)
                filler(1)
                afd_ps = tp.tile([F_ND, 128], f32, name=f"afd_ps{p}",
                                 tag="tp")
                nc.tensor.matmul(afd_ps[:], afd[:], ident,
                                 is_transpose=True, start=True, stop=True)
                nc.vector.tensor_copy(afdt[p][:], afd_ps[:])

                # heads -> hcT -> final_Q^T per group
                fqp = tp.tile([128, 2 * V], f32, name=f"fqp{p}", tag="tp")
                for g2 in range(2):
                    g = 2 * p + g2
                    hq_ps = tp.tile([128, 64], f32, name=f"hq_ps{g}",
                                    tag="tp")
                    nc.tensor.matmul(
                        hq_ps[:],
                        ha_sb[p][g2 * 64:(g2 + 1) * 64,
                                 g2 * 128:(g2 + 1) * 128],
                        identpad[g2 * 64:(g2 + 1) * 64, :],
                        is_transpose=True, start=True, stop=False,
                        skip_group_check=True)
                    nc.tensor.matmul(
                        hq_ps[:], wnsv,
                        afdt[p][:, g2 * 64:(g2 + 1) * 64],
                        start=False, stop=True, skip_group_check=True)
                    hqs = S(f"hqs{g}", [128, 64])
                    nc.vector.tensor_tensor(hqs[:], hq_ps[:], hsel,
                                            OP.mult)
                    nc.vector.tensor_reduce(
                        hct[p][:, g2 * V:(g2 + 1) * V],
                        hqs.rearrange("q (hh v) -> q v hh", v=V),
                        AX.X, OP.add)
                    filler(1)
                filler(3)
                nc.tensor.matmul(fqp[:], wout, hct[p][:],
                                 start=True, stop=True)
                nc.vector.tensor_copy(fqt[p][:], fqp[:])
                for g2 in range(2):
                    g = 2 * p + g2
                    nc.vector.tensor_copy(
                        fq32g[g][:, g * 8:(g + 1) * 8],
                        fqp[:, g2 * V:(g2 + 1) * V])

                # block-diag fw
                fw_ps = tp.tile([F_ND, 2 * V], f32, name=f"fw_ps{p}",
                                tag="tp")
                nc.tensor.matmul(fw_ps[:], wnslt, fqt[p][:],
                                 start=True, stop=True)
                fw_sbt = S(f"fw_sbt{p}", [F_ND, 2 * V])
                nc.vector.tensor_copy(fw_sbt[:], fw_ps[:])
                filler(2)
                fwr_ps = tp.tile([128, 2 * V], f32, name=f"fwr_ps{p}",
                                 tag="tp")
                nc.tensor.matmul(fwr_ps[:], repl, fw_sbt[:],
                                 start=True, stop=True)
                for g2 in range(2):
                    g = 2 * p + g2
                    gsl = slice(g2 * 64, (g2 + 1) * 64)
                    vsl = slice(g2 * V, (g2 + 1) * V)
                    nc.vector.tensor_tensor(
                        bdfw32[g][:, g * 8:(g + 1) * 8],
                        fwr_ps[gsl, vsl], fwsel[gsl, vsl], OP.mult)

            def fill_ndfn(p):
                # rhsha region 2 = ndf in natural [n, (g2,v,f)] layout,
                # derived from ndftm by 16 PE transposes (saves 1MB DMA)
                for c in range(8):
                    nt_ps = tp.tile([128, 128], f32r,
                                    name=f"nt_ps{p}_{c}", tag="tp")
                    for g2 in range(2):
                        nc.tensor.matmul(
                            nt_ps[:, g2 * 64:(g2 + 1) * 64],
                            ndftm[2 * p + g2][0:64,
                                              c * 128:(c + 1) * 128],
                            identpad_r[:],
                            is_transpose=True, start=True, stop=True,
                            skip_group_check=True)
                    nc.vector.tensor_copy(
                        rhsha[p][:, 2 * N + c * 128:2 * N + (c + 1) * 128],
                        nt_ps[:])
                    filler(1)

            phase_C(0)
            filler(1)
            phase_C(1)
            fill_ndfn(0)
            phase_C(2)
            filler(1)
            phase_C(3)
            fill_ndfn(1)
            phase_T(0)
            filler(1)
            phase_T(1)
            filler(4)
            phase_H(0)
            filler(2)
            phase_H(1)
            filler(2)
            phase_sm(0)
            filler(2)
            phase_sm(1)
            filler(2)

            # ======== phase E: batched logits for all 4 groups ===========
            # lg32[8g+v, n] = logits of (group g, vehicle v): group g's
            # stationaries are zero-padded to 32 cols at offset 8g so all
            # 16 matmuls accumulate into one [32, N] PSUM tile.
            lg32 = acc.tile([32, N], f32, name="lg32", tag="acc")
            for half in range(2):
                sl = slice(half * 512, (half + 1) * 512)
                for g in range(G):
                    nc.tensor.matmul(
                        lg32[:, sl], bdfw32[g][:], ndftm[g][0:64, sl],
                        start=(g == 0), stop=False, skip_group_check=True)
                for g in range(G):
                    nc.tensor.matmul(
                        lg32[:, sl], fq32g[g][:],
                        lt[:, g * N:(g + 1) * N][:, sl],
                        start=False, stop=(g == G - 1),
                        skip_group_check=True)
            # lgf = 10*tanh(x/sqrt(D)) + mask, in halves so the DVE
            # mask-add of half 0 overlaps the ACT tanh of half 1
            for half in range(2):
                sl = slice(half * 512, (half + 1) * 512)
                nc.scalar.activation(u32[:, sl], lg32[:, sl], AF_TANH,
                                     scale=float(1.0 / np.sqrt(D)))
                nc.vector.scalar_tensor_tensor(
                    lgf32[:, sl], u32[:, sl], TANH_CLIP, mbs32[:, sl],
                    op0=OP.mult, op1=OP.add)

            # ============ epilogue: batched flat log-softmax/argmax ======
            rs32 = S("rs32e", [32, 1])
            expf = S("expfe", [32, N])
            nc.scalar.activation(expf[:], lgf32[:], AF_EXP,
                                 accum_out=rs32[:])
            mx8 = S("mx8e", [32, 8])
            ix8 = S("ix8e", [32, 8], dt.uint32)
            nc.vector.max_with_indices(mx8[:], ix8[:], lgf32[:])
            idxf = S("idxfe", [32, 1])
            nc.vector.tensor_copy(idxf[:], ix8[:, 0:1])
            cand = S("cande", [32, 1])
            nc.vector.tensor_tensor(cand[:], c8k32, idxf[:], OP.subtract)

            rmt_ps = tp.tile([1, 32], f32, name="rmt_ps", tag="tp")
            nc.tensor.matmul(rmt_ps[:], mx8[:, 0:1], ident[0:32, 0:32],
                             is_transpose=True, start=True, stop=True)
            rmt = S("rmte", [1, 32])
            nc.vector.tensor_copy(rmt[:], rmt_ps[:])
            rst_ps = tp.tile([1, 32], f32, name="rst_ps", tag="tp")
            nc.tensor.matmul(rst_ps[:], rs32[:], ident[0:32, 0:32],
                             is_transpose=True, start=True, stop=True)
            rst = S("rste", [1, 32])
            nc.vector.tensor_copy(rst[:], rst_ps[:])
            cdt_ps = tp.tile([1, 32], f32, name="cdt_ps", tag="tp")
            nc.tensor.matmul(cdt_ps[:], cand[:], ident[0:32, 0:32],
                             is_transpose=True, start=True, stop=True)
            cdt = S("cdte", [1, 32])
            nc.vector.tensor_copy(cdt[:], cdt_ps[:])

            mt4 = S("mt4e", [1, G])
            nc.vector.tensor_reduce(mt4[:],
                                    rmt.rearrange("o (g v) -> o g v", g=G),
                                    AX.X, OP.max)
            s4 = S("s4e", [1, G])
            nc.vector.tensor_reduce(s4[:],
                                    rst.rearrange("o (g v) -> o g v", g=G),
                                    AX.X, OP.add)
            em4 = S("em4e", [1, G])
            nc.scalar.activation(em4[:], mt4[:], AF_EXP, scale=-1.0)
            s4p = S("s4pe", [1, G])
            nc.vector.tensor_tensor(s4p[:], s4[:], em4[:], OP.mult)
            lns4 = S("lns4e", [1, G])
            nc.scalar.activation(lns4[:], s4p[:], AF_LN)
            prob4 = S("prob4e", [1, G])
            nc.vector.reciprocal(prob4[:], s4p[:])
            mtb = (mt4.unsqueeze(2).broadcast_to([1, G, V]))
            eq = S("eqe", [1, 32])
            nc.vector.tensor_tensor(
                eq.rearrange("o (g v) -> o g v", g=G),
                rmt.rearrange("o (g v) -> o g v", g=G), mtb, OP.is_equal)
            cs = S("cse", [1, 32])
            nc.vector.tensor_tensor(cs[:], eq[:], cdt[:], OP.mult)
            cm4 = S("cm4e", [1, G])
            nc.vector.tensor_reduce(cm4[:],
                                    cs.rearrange("o (g v) -> o g v", g=G),
                                    AX.X, OP.max)
            res16 = S("res16e", [1, 4 * G])
            nc.vector.tensor_scalar(res16[:, 0:16:4], cm4[:], -1.0, 8192.0,
                                    OP.mult, OP.add)
            nc.vector.tensor_scalar_mul(res16[:, 1:16:4], lns4[:], -1.0)
            nc.vector.tensor_tensor(res16[:, 2:16:4], prob4[:], lns4[:],
                                    OP.mult)
            nc.vector.memset(res16[:, 3:16:4], 0.0)
            nc.sync.dma_start(
                res_out.ap().rearrange("a b -> (a b)").unsqueeze(0),
                res16[:])

    nc.compile()
    return nc


def _get_program():
    if "nc" not in _PROGRAM_CACHE:
        _PROGRAM_CACHE["nc"] = _build_program()
    return _PROGRAM_CACHE["nc"]


# --------------------------------------------------------------------------
# Host-side sharding / layout prep
# --------------------------------------------------------------------------

def _make_in_maps(inputs):
    gk = np.asarray(inputs["glimpse_K_static"], dtype=np.float32)
    gv = np.asarray(inputs["glimpse_V_static"], dtype=np.float32)
    lk = np.asarray(inputs["logit_K_static"], dtype=np.float32)
    ndf = np.asarray(inputs["node_dynamic_features"], dtype=np.float32)
    vdf = np.asarray(inputs["vehicle_dynamic_features"], dtype=np.float32)
    fc = np.asarray(inputs["fixed_context"], dtype=np.float32)
    msk = np.asarray(inputs["feasibility_mask"])
    w_cs = np.asarray(inputs["W_cs"], dtype=np.float32)
    w_ns = np.asarray(inputs["W_ns"], dtype=np.float32)
    w_out = np.asarray(inputs["W_out"], dtype=np.float32)

    in_maps = []
    for c in range(NCORES):
        bs = slice(c * G, (c + 1) * G)
        kt = np.ascontiguousarray(
            gk[:, bs].transpose(1, 0, 3, 2).reshape(G, 128, N))
        lt = np.ascontiguousarray(
            lk[bs].transpose(0, 2, 1).transpose(1, 0, 2).reshape(128, G * N))
        vn = gv[:, bs].transpose(1, 2, 0, 3).reshape(G, N, 128)
        nd = ndf[bs]                                   # [G, V, N, F]
        ndftm = np.zeros((G, 72, N), dtype=np.float32)
        ndftm[:, 0:64, :] = nd.transpose(0, 1, 3, 2).reshape(G, 64, N)
        mbx = (msk[bs].astype(np.float32) - 1.0) * MASK_BIG   # [G, V, N]
        ndftm[:, 64:72, :] = mbx
        mbs32 = np.ascontiguousarray(mbx.reshape(32, N))
        rhsha = np.empty((NPAIR, 128, 16, 128), dtype=np.float32)
        for p in range(NPAIR):
            for g2 in range(2):
                rhsha[p, :, g2 * 8:(g2 + 1) * 8, :] = (
                    vn[2 * p + g2].reshape(8, 128, 128).transpose(1, 0, 2))
        rhsha = np.ascontiguousarray(rhsha.reshape(NPAIR, 128, 2 * N))
        wpack = np.zeros((128, 564), dtype=np.float32)
        wpack[:, 0:128] = w_cs[:D]
        wpack[:, 128:256] = w_out
        wpack[0:F_ND, 256:384] = w_ns[:, 0:D]
        wpack[0:F_V, 384:512] = w_cs[D:]
        wpack[:, 512:516] = fc[bs].T
        wpack[0:F_V, 516:548] = vdf[bs].transpose(2, 0, 1).reshape(F_V, 32)
        wpack[:, 548:556] = w_ns[:, D:2 * D].T
        wpack[:, 556:564] = w_ns[:, 2 * D:3 * D].T
        in_maps.append({
            "kt_in": kt,
            "lt_in": lt,
            "rhsha_in": rhsha,
            "ndftm_in": ndftm,
            "mbs32_in": mbs32,
            "wpack_in": wpack,
        })
    return in_maps


def _postprocess(res_list):
    out = np.concatenate(res_list, axis=0)              # [B, 4]
    a = out[:, 0]
    lp = out[:, 1].astype(np.float32)
    ent = out[:, 2].astype(np.float32)
    sel_vec = (a.astype(np.float32) / np.float32(N)).astype(np.float32)
    sel_node = (np.round(a).astype(np.int64) % N).astype(np.int32)
    return sel_vec, sel_node, lp, ent


LAST_RESULTS = None
ENABLE_LDW_OPT = True
_LDW_PATCHED = False


def _patch_ldw_opt():
    """Flip walrus --enable-ldw-opt (elides redundant PE weight loads)."""
    global _LDW_PATCHED
    if _LDW_PATCHED or not ENABLE_LDW_OPT:
        return
    import concourse.bass_utils as bu
    orig = bu.run_command

    def patched(argv, **kw):
        argv = ["--enable-ldw-opt=true" if a == "--enable-ldw-opt=false"
                else a for a in argv]
        return orig(argv, **kw)

    bu.run_command = patched
    _LDW_PATCHED = True


def _run(inputs, trace=False):
    global LAST_RESULTS
    _patch_ldw_opt()
    from concourse.bass_utils import run_bass_kernel_spmd
    nc = _get_program()
    in_maps = _make_in_maps(inputs)
    res = run_bass_kernel_spmd(nc, in_maps, list(range(NCORES)), trace=trace)
    LAST_RESULTS = res
    return _postprocess([res.results[c]["res_out"] for c in range(NCORES)])


def kernel(**inputs):
    return _run(inputs, trace=False)


# revision 24
# speedup vs baseline: 1.3446x; 1.3446x over previous
"""Trainium2 Bass kernel for nn_Agent_57732950393167 (ragged_sequence).

Strategy
--------
Data-parallel over batches: 32 batches / 8 cores = 4 batches ("groups" g)
per core, each with V=8 vehicles -> 32 vehicles/core.

Key restructurings (vs. the reference):
 * nde = ndf @ W_ns ([T,N,384], 402MB) is NEVER materialized.  It is rank-8
   in the feature dim, so its three uses are folded:
     - K-part: compat_dyn[t,h,n] = sum_f ndf[t,n,f] * qw[t][f,h],
       qw = (Q/4) . W_nsK head-blocks  (an [8,8] matrix per vehicle)
     - V-part: heads_dyn = (sum_n attn*ndf) @ W_nsV  (attn-weighted feature
       sums AF[t,h,f], an [8,8] per vehicle)
     - L-part: logits_dyn[t,n] = sum_f ndf[t,n,f] * (W_nsL . final_Q[t])
 * Big matmuls batch the 64 (vehicle,head) rows of a batch-pair into one
   PE pass using block-diagonal stationary matrices (built on device via a
   replication matmul + select-mask; PE partition bases stay 32-aligned).
 * Every DMA ships in its exact SBUF destination layout (host prep), so
   each transfer is per-partition contiguous, and ALL input transfers
   ride ONE HWDGE ring (sync) issued in consumption order: a single
   queue drains strictly FIFO at the full per-NC HBM rate, so arrival
   order is exactly the issue order.  (Splitting across rings shares
   bandwidth per packet-size, not priority -- measured 3x starvation.)
 * PE program order matches arrival order (prime warm-up -> phase A ->
   C0..C3 -> T0 T1 -> H0 H1 -> sm0 sm1 -> batched logits -> epilogue),
   with junk "filler" matmuls at phase boundaries so the PE activity
   monitor (HAM) keeps the 2.4 GHz clock instead of the idle-throttled
   1.2 GHz.  (PE transposes do NOT count as HAM activity, but
   interleaving fillers INTO transpose blocks serializes the PE --
   fillers only belong at boundaries.)
 * Softmax runs unnormalized (no max shift -- |scores| < ~15); the 1/sum
   is folded into the single heads-PSUM rescale via a [128,1] per-pair
   reciprocal vector.
 * Logits phase is fully batched: all 4 groups' logits accumulate into
   one [32, N] PSUM tile via zero-padded stationaries (group g's columns
   at 8g..8g+8), then tanh+mask in 512-halves so DVE overlaps ACT.
 * Epilogue: one [32,N] max_with_indices / one Exp; lp = M - ln(S),
   prob = 1/(S*e^-M) via DVE reciprocal (the Ln table reload is hidden
   behind the DVE argmax chain by engine in-order execution).
 * Numerics must stay f32/f32r everywhere: pre-tanh logits are ~0.05 and
   the tightest batch has a 2e-4 top-2 gap, so bf16 anywhere in the
   compat/heads/logits path flips the argmax.

log(mask) is approximated by MASK_BIG*(mask-1) with MASK_BIG=50 (exact to
~1e-9 relative in the final softmax sums).
"""

import numpy as np

B, N, D, H, V = 32, 1024, 128, 8, 8
KS = D // H            # 16
F_V = 4
F_ND = 8
TANH_CLIP = 10.0
MASK_BIG = 50.0
NCORES = 8
G = B // NCORES        # 4 groups (batches) per core
NPAIR = G // 2         # 2 batch-pairs per core

_PROGRAM_CACHE = {}

CP_W = 849             # const pack width


def _build_const_pack():
    """[128, CP_W] f32: all device constants, one DMA."""
    KSl = KS
    cp = np.zeros((128, CP_W), dtype=np.float32)
    cp[:, 0:128] = np.eye(128, dtype=np.float32)                  # ident
    cp[:, 128:192] = np.tile(np.eye(64, dtype=np.float32), (2, 1))  # identpad
    cp[0:F_ND, 192:320] = np.tile(np.eye(F_ND, dtype=np.float32),
                                  (1, 16))                        # repl
    dm = np.zeros((128, 128), dtype=np.float32)                   # diagmask
    for p in range(128):
        g2, hv = divmod(p, 64)
        a = g2 * 8 + (hv % 8)
        dm[p, a * 8:(a + 1) * 8] = 1.0
    cp[:, 320:448] = dm
    bd = np.zeros((128, 128), dtype=np.float32)                   # bdsel
    for p in range(128):
        a = p // 8
        g2, v = divmod(a, 8)
        bd[p, g2 * 64 + v:g2 * 64 + 64:8] = 1.0
    cp[:, 448:576] = bd
    fw = np.zeros((128, 16), dtype=np.float32)                    # fwsel
    for p in range(128):
        fw[p, p // 8] = 1.0
    cp[:, 576:592] = fw
    hs = np.zeros((128, 64), dtype=np.float32)                    # hsel
    for hk in range(128):
        h = hk // KSl
        hs[hk, h * 8:(h + 1) * 8] = 1.0
    cp[:, 592:656] = hs
    hb = np.zeros((128, 128), dtype=np.float32)                   # hselb
    for d in range(128):
        h = d // KSl
        for g2 in range(2):
            hb[d, g2 * 64 + h * V:g2 * 64 + (h + 1) * V] = 1.0
    cp[:, 656:784] = hb
    rb = np.zeros((V, H * V), dtype=np.float32)                   # replbig
    for v in range(V):
        rb[v, v::V] = 1.0
    cp[0:V, 784:848] = rb
    cp[0:32, 848] = 8192.0 - 1024.0 * (np.arange(32) % 8)         # c8k32
    return cp


# --------------------------------------------------------------------------
# Device program
# --------------------------------------------------------------------------

def _build_program():
    import contextlib

    import concourse.bacc as bacc
    import concourse.tile as tile
    import concourse.mybir as mybir

    dt = mybir.dt
    f32 = dt.float32
    f32r = dt.float32r
    AF_EXP = mybir.ActivationFunctionType.Exp
    AF_LN = mybir.ActivationFunctionType.Ln
    AF_TANH = mybir.ActivationFunctionType.Tanh
    AF_IDENT = mybir.ActivationFunctionType.Identity
    AF_COPY = mybir.ActivationFunctionType.Copy
    OP = mybir.AluOpType
    AX = mybir.AxisListType

    nc = bacc.Bacc("TRN2", target_bir_lowering=False, debug=False,
                   num_devices=NCORES)

    # ---- external inputs (per-core shards, host-prepped layouts) ----
    kt_in = nc.dram_tensor("kt_in", [G, 128, N], f32r, kind="ExternalInput")
    lt_in = nc.dram_tensor("lt_in", [128, G * N], f32r,
                           kind="ExternalInput")
    rhsha_in = nc.dram_tensor("rhsha_in", [NPAIR, 128, 3 * N], f32r,
                              kind="ExternalInput")
    ndftm_in = nc.dram_tensor("ndftm_in", [G, 72, N], f32r,
                              kind="ExternalInput")
    mbs32_in = nc.dram_tensor("mbs32_in", [32, N], f32r,
                              kind="ExternalInput")
    wpack_in = nc.dram_tensor("wpack_in", [128, 564], f32,
                              kind="ExternalInput")

    res_out = nc.dram_tensor("res_out", [G, 4], f32, kind="ExternalOutput")

    cpack_c = nc.inline_tensor(_build_const_pack(), name="cpack_c")

    with tile.TileContext(nc) as tc:
        with contextlib.ExitStack() as ctx:
            sb = ctx.enter_context(tc.tile_pool(name="sb", bufs=1))
            scr = ctx.enter_context(tc.tile_pool(name="scr", bufs=4))
            acc = ctx.enter_context(
                tc.tile_pool(name="acc", bufs=2, space="PSUM"))
            tp = ctx.enter_context(
                tc.tile_pool(name="tp", bufs=3, space="PSUM"))
            flp = ctx.enter_context(
                tc.tile_pool(name="flp", bufs=1, space="PSUM"))

            def P(name, shape, dtype=f32):
                return sb.tile(shape, dtype, name=name, tag=name)

            def S(name, shape, dtype=f32):
                if shape[-1] >= 512:
                    return scr.tile(shape, dtype, name=name, tag="sbig",
                                    bufs=4)
                return scr.tile(shape, dtype, name=name, tag="ssml", bufs=8)

            # ================= persistent SBUF tiles =================
            kt = P("kt", [128, G * N], f32r)       # K^T  (g,n) cols
            lt = P("lt", [128, G * N], f32r)       # logitK^T
            ndftm = [P(f"ndftm{g}", [72, N], f32r) for g in range(G)]
            rhsha = [P(f"rhsha{p}", [128, 3 * N], f32r) for p in range(NPAIR)]
            attnt = [P(f"attntp{p}", [128, N]) for p in range(NPAIR)]
            attnnt = [P(f"attnnt{p}", [128, 8 * 128], f32r)
                      for p in range(NPAIR)]
            cpack = P("cpack", [128, CP_W])
            wpack = P("wpack", [128, 564])
            wpackr = P("wpackr", [128, 16], f32r)
            replbig_r = P("replbig_r", [V, H * V], f32r)
            mbs32 = P("mbs32", [32, N], f32r)      # 50*(mask-1), rows (g,v)
            fctq = P("fctq", [128, G])
            queryt = P("queryt", [128, G * V])     # 0.25-scaled query^T
            blockq = [P(f"blockq{p}", [128, 128], f32r) for p in range(NPAIR)]
            bdq72 = [P(f"bdq72_{g}", [72, 64], f32r) for g in range(G)]
            ha_sb = [P(f"hasb{p}", [128, 384]) for p in range(NPAIR)]
            afdt = [P(f"afdt{p}", [F_ND, 128]) for p in range(NPAIR)]
            hct = [P(f"hctp{p}", [128, 2 * V]) for p in range(NPAIR)]
            fqt = [P(f"fqt{p}", [128, 2 * V], f32r) for p in range(NPAIR)]
            fq32g = [P(f"fq32g{g}", [128, 32], f32r) for g in range(G)]
            bdfw32 = [P(f"bdfw32_{g}", [64, 32], f32r) for g in range(G)]
            lgf32 = P("lgf32", [32, N])
            u32 = P("u32", [32, N])
            rinv_p = [P(f"rinvp{p}", [128, 1]) for p in range(NPAIR)]
            prime_sb = P("prime_sb", [128, 512])

            # ================= DMA issues =================
            # Everything rides ONE HWDGE ring (sync) in exact consumption
            # order: a single queue gets the full per-NC HBM bandwidth and
            # drains strictly FIFO, so arrival order == this issue order.
            # (Queues round-robin per packet, so splitting rings would give
            # bandwidth by packet size, not priority; and the tile
            # scheduler reorders dep-free instructions, so ordering must be
            # expressed by issue sequence on one engine.)
            nc.sync.dma_start(wpack[:], wpack_in.ap())
            nc.sync.dma_start(cpack[:], cpack_c.ap())
            for g in range(G):
                nc.sync.dma_start(kt[:, g * N:(g + 1) * N], kt_in[g])
                nc.sync.dma_start(ndftm[g][:], ndftm_in[g])
            nc.sync.dma_start(rhsha[0][:], rhsha_in[0])
            nc.sync.dma_start(rhsha[1][:], rhsha_in[1])
            nc.sync.dma_start(lt[:, 0:2 * N], lt_in.ap()[:, 0:2 * N])
            nc.sync.dma_start(lt[:, 2 * N:4 * N],
                              lt_in.ap()[:, 2 * N:4 * N])
            nc.sync.dma_start(mbs32[:], mbs32_in.ap())

            # ================= small setup on DVE ==================
            nc.vector.memset(prime_sb[:], 0.0)
            for g in range(G):
                # f32r tiles cannot be memset directly; copy zeros instead
                nc.vector.tensor_copy(fq32g[g][:], prime_sb[:, 0:32])
                nc.vector.tensor_copy(bdfw32[g][:], prime_sb[0:64, 0:32])

            # PE warm-up: back-to-back matmuls so the HAM un-throttles the
            # PE clock right as real data lands; the tail ones read the
            # first real arrivals so the chain stretches to meet them.
            for i in range(4):
                prime_ps = tp.tile([128, 512], f32, name=f"prime{i}",
                                   tag="tp")
                nc.tensor.matmul(prime_ps[:], prime_sb[:, 0:128],
                                 prime_sb[:], start=True, stop=True,
                                 skip_group_check=True)
            prime_k = tp.tile([128, 512], f32, name="prime_k", tag="tp")
            nc.tensor.matmul(prime_k[:], kt[:, 0:128], kt[:, 0:512],
                             start=True, stop=True, skip_group_check=True)
            prime_n = tp.tile([72, 512], f32, name="prime_n", tag="tp")
            nc.tensor.matmul(prime_n[:], ndftm[0][:, 0:72],
                             ndftm[0][:, 0:512],
                             start=True, stop=True, skip_group_check=True)

            # HAM keep-warm fillers: junk matmuls into a dedicated PSUM
            # bank, inserted at phase boundaries so the PE activity
            # monitor never sees an idle window and holds the 2.4 GHz
            # clock.  f32r with a 256-wide moving operand = 107 ns each
            # warm; the kt stationary is reused so LDW is elided.
            fl_ps = flp.tile([128, 256], f32, name="fl_ps", tag="fl")

            def filler(n, early=False):
                for _ in range(n):
                    if early:
                        nc.tensor.matmul(fl_ps[:, 0:128],
                                         prime_sb[:, 0:128],
                                         prime_sb[:, 0:128],
                                         start=True, stop=True,
                                         skip_group_check=True)
                    else:
                        nc.tensor.matmul(fl_ps[:], kt[:, 0:128],
                                         kt[:, 0:256],
                                         start=True, stop=True,
                                         skip_group_check=True)

            # const/weight slices
            ident = cpack[:, 0:128]
            identpad = cpack[:, 128:192]
            repl = cpack[0:F_ND, 192:320]
            diagmask = cpack[:, 320:448]
            bdsel = cpack[:, 448:576]
            fwsel = cpack[:, 576:592]
            hsel = cpack[:, 592:656]
            hselb = cpack[:, 656:784]
            replbig = cpack[0:V, 784:848]
            c8k32 = cpack[0:32, 848:849]
            wcs_hi = wpack[:, 0:128]
            wout = wpack[:, 128:256]
            wnsv = wpack[0:F_ND, 256:384]
            wcs_lo = wpack[0:F_V, 384:512]
            fct = wpack[:, 512:516]
            vdft = wpack[0:F_V, 516:548]
            nc.vector.tensor_copy(wpackr[:], wpack[:, 548:564])
            identpad_r = P("identpad_r", [64, 64], f32r)
            nc.vector.tensor_copy(identpad_r[:], cpack[0:64, 128:192])
            wnskt = wpackr[:, 0:F_ND]
            wnslt = wpackr[:, F_ND:2 * F_ND]

            # fctq = 0.25*fc^T
            nc.vector.tensor_scalar_mul(fctq[:], fct, 0.25)
            nc.vector.tensor_copy(replbig_r[:], replbig)

            # ================= phase A: query / qw smalls =================
            fct8a = S("fct8a", [128, G * V])
            for g in range(G):
                nc.scalar.activation(fct8a[:, g * V:(g + 1) * V],
                                     ident[:, 0:V], AF_IDENT,
                                     bias=fct[:, g:g + 1], scale=0.0)
            qt_ps = tp.tile([128, G * V], f32, name="qt_ps", tag="tp")
            nc.tensor.matmul(qt_ps[:], wcs_hi, fct8a[:],
                             start=True, stop=False, skip_group_check=True)
            nc.tensor.matmul(qt_ps[:], wcs_lo, vdft,
                             start=False, stop=True, skip_group_check=True)
            filler(3, early=True)
            for g in range(G):
                # queryt = 0.25*(cur + fc)
                nc.scalar.activation(queryt[:, g * V:(g + 1) * V],
                                     qt_ps[:, g * V:(g + 1) * V],
                                     AF_IDENT, bias=fctq[:, g:g + 1],
                                     scale=0.25)

            for p in range(NPAIR):
                # blockq[d, (g2,h,v)] = queryt[d, (g,v)] * (h == d//16)
                qview = (queryt[:, 2 * p * V:(2 * p + 2) * V]
                         .rearrange("d (g2 v) -> d g2 v", g2=2)
                         .unsqueeze(2).broadcast_to([128, 2, H, V]))
                nc.vector.tensor_tensor(
                    blockq[p].rearrange("d (g2 h v) -> d g2 h v", g2=2, h=H),
                    qview, hselb.rearrange("d (g2 h v) -> d g2 h v",
                                           g2=2, h=H),
                    OP.mult)
                # qw_all[f, (g2,h,v)] then replicate+mask into block-diag
                qw_ps = tp.tile([F_ND, 128], f32, name=f"qw_ps{p}", tag="tp")
                nc.tensor.matmul(qw_ps[:], wnskt, blockq[p][:],
                                 start=True, stop=True)
                qw_sbt = S(f"qw_sbt{p}", [F_ND, 128])
                nc.vector.tensor_copy(qw_sbt[:], qw_ps[:])
                qwr_ps = tp.tile([128, 128], f32, name=f"qwr_ps{p}", tag="tp")
                nc.tensor.matmul(qwr_ps[:], repl, qw_sbt[:],
                                 start=True, stop=True)
                filler(2, early=True)
                for g2 in range(2):
                    g = 2 * p + g2
                    gsl = slice(g2 * 64, (g2 + 1) * 64)
                    nc.vector.tensor_tensor(bdq72[g][0:64, :],
                                            qwr_ps[gsl, gsl],
                                            bdsel[gsl, gsl], OP.mult)
                    nc.vector.tensor_copy(bdq72[g][64:72, :], replbig)

            # ===== phases C/T/H/sm as functions, issued in consumption
            # order so the PE stream matches the single-ring DMA arrival
            # order: C0, C1, T0, H0, C2, C3, sm0, T1, H1, sm1, E.
            def phase_C(g):
                p, g2 = divmod(g, 2)
                gsl = slice(g2 * 64, (g2 + 1) * 64)
                compat = acc.tile([64, N], f32, name=f"compat{g}",
                                  tag="acc")
                # dyn + mask in one 72-row contraction; static after
                for half in range(2):
                    sl = slice(half * 512, (half + 1) * 512)
                    nc.tensor.matmul(
                        compat[:, sl], bdq72[g][:], ndftm[g][:, sl],
                        start=True, stop=False, skip_group_check=True)
                for half in range(2):
                    sl = slice(half * 512, (half + 1) * 512)
                    nc.tensor.matmul(
                        compat[:, sl], blockq[p][:, gsl],
                        kt[:, g * N:(g + 1) * N][:, sl],
                        start=False, stop=True, skip_group_check=True)
                # unnormalized exp into the pair tile (|compat| < ~15)
                rsum = S(f"rsum{g}", [64, 1])
                nc.scalar.activation(attnt[p][gsl, :], compat[:],
                                     AF_EXP, accum_out=rsum[:])
                nc.vector.reciprocal(rinv_p[p][gsl, :], rsum[:])

            def phase_T(p):
                # attn^T -> attn_n: [128,128] transposes, paired copies
                for c2 in range(4):
                    at_ps = tp.tile([128, 256], f32,
                                    name=f"at_ps{p}_{c2}", tag="tp")
                    for j in range(2):
                        c = 2 * c2 + j
                        nc.tensor.matmul(
                            at_ps[:, j * 128:(j + 1) * 128],
                            attnt[p][:, c * 128:(c + 1) * 128],
                            ident,
                            is_transpose=True,
                            start=True, stop=True,
                            skip_group_check=True)
                    dst = (attnnt[p]
                           .rearrange("q (c w) -> q c w", w=128)
                           [:, 2 * c2:2 * c2 + 2, :])
                    src_ap = at_ps.rearrange("q (j w) -> q j w", j=2)
                    if c2 % 2 == 0:
                        nc.scalar.activation(dst, src_ap, AF_COPY)
                    else:
                        nc.vector.tensor_copy(dst, src_ap)

            def phase_H(p):
                # heads+AF over the 3 contiguous rhsha regions (2-dim
                # free AP); normalization via rinv in the PSUM->SBUF move
                ha_ps = tp.tile([128, 384], f32, name=f"ha_ps{p}", tag="tp")
                rh = rhsha[p].rearrange("q (r w) -> q r w", r=3)
                for c in range(8):
                    nc.tensor.matmul(ha_ps[:],
                                     attnnt[p][:, c * 128:(c + 1) * 128],
                                     rh[:, :, c * 128:(c + 1) * 128],
                                     start=(c == 0), stop=(c == 7))
                nc.vector.tensor_scalar_mul(ha_sb[p][:], ha_ps[:],
                                            rinv_p[p][:])

            def phase_sm(p):
                # AF diag-extract -> AFd [128, F] -> AFd^T
                aftmp = S(f"aftmp{p}", [128, 128])
                nc.vector.tensor_tensor(aftmp[:], ha_sb[p][:, 256:384],
                                        diagmask, OP.mult)
                afd = S(f"afd{p}", [128, F_ND])
                nc.vector.tensor_reduce(
                    afd[:], aftmp.rearrange("q (a f) -> q f a", f=F_ND),
                    AX.X, OP.add)
                filler(1)
                afd_ps = tp.tile([F_ND, 128], f32, name=f"afd_ps{p}",
                                 tag="tp")
                nc.tensor.matmul(afd_ps[:], afd[:], ident,
                                 is_transpose=True, start=True, stop=True)
                nc.vector.tensor_copy(afdt[p][:], afd_ps[:])

                # heads -> hcT -> final_Q^T per group
                fqp = tp.tile([128, 2 * V], f32, name=f"fqp{p}", tag="tp")
                for g2 in range(2):
                    g = 2 * p + g2
                    hq_ps = tp.tile([128, 64], f32, name=f"hq_ps{g}",
                                    tag="tp")
                    nc.tensor.matmul(
                        hq_ps[:],
                        ha_sb[p][g2 * 64:(g2 + 1) * 64,
                                 g2 * 128:(g2 + 1) * 128],
                        identpad[g2 * 64:(g2 + 1) * 64, :],
                        is_transpose=True, start=True, stop=False,
                        skip_group_check=True)
                    nc.tensor.matmul(
                        hq_ps[:], wnsv,
                        afdt[p][:, g2 * 64:(g2 + 1) * 64],
                        start=False, stop=True, skip_group_check=True)
                    hqs = S(f"hqs{g}", [128, 64])
                    nc.vector.tensor_tensor(hqs[:], hq_ps[:], hsel,
                                            OP.mult)
                    nc.vector.tensor_reduce(
                        hct[p][:, g2 * V:(g2 + 1) * V],
                        hqs.rearrange("q (hh v) -> q v hh", v=V),
                        AX.X, OP.add)
                filler(3)
                nc.tensor.matmul(fqp[:], wout, hct[p][:],
                                 start=True, stop=True)
                nc.vector.tensor_copy(fqt[p][:], fqp[:])
                for g2 in range(2):
                    g = 2 * p + g2
                    nc.vector.tensor_copy(
                        fq32g[g][:, g * 8:(g + 1) * 8],
                        fqp[:, g2 * V:(g2 + 1) * V])

                # block-diag fw
                fw_ps = tp.tile([F_ND, 2 * V], f32, name=f"fw_ps{p}",
                                tag="tp")
                nc.tensor.matmul(fw_ps[:], wnslt, fqt[p][:],
                                 start=True, stop=True)
                fw_sbt = S(f"fw_sbt{p}", [F_ND, 2 * V])
                nc.vector.tensor_copy(fw_sbt[:], fw_ps[:])
                filler(2)
                fwr_ps = tp.tile([128, 2 * V], f32, name=f"fwr_ps{p}",
                                 tag="tp")
                nc.tensor.matmul(fwr_ps[:], repl, fw_sbt[:],
                                 start=True, stop=True)
                for g2 in range(2):
                    g = 2 * p + g2
                    gsl = slice(g2 * 64, (g2 + 1) * 64)
                    vsl = slice(g2 * V, (g2 + 1) * V)
                    nc.vector.tensor_tensor(
                        bdfw32[g][:, g * 8:(g + 1) * 8],
                        fwr_ps[gsl, vsl], fwsel[gsl, vsl], OP.mult)

            phase_C(0)
            filler(1)
            phase_C(1)
            filler(1)
            phase_C(2)
            filler(1)
            phase_C(3)
            filler(1)
            phase_T(0)
            filler(1)
            phase_T(1)
            filler(4)
            phase_H(0)
            filler(2)
            phase_H(1)
            filler(2)
            phase_sm(0)
            filler(2)
            phase_sm(1)
            filler(2)

            # ======== phase E: batched logits for all 4 groups ===========
            # lg32[8g+v, n] = logits of (group g, vehicle v): group g's
            # stationaries are zero-padded to 32 cols at offset 8g so all
            # 16 matmuls accumulate into one [32, N] PSUM tile.
            lg32 = acc.tile([32, N], f32, name="lg32", tag="acc")
            for half in range(2):
                sl = slice(half * 512, (half + 1) * 512)
                for g in range(G):
                    nc.tensor.matmul(
                        lg32[:, sl], bdfw32[g][:], ndftm[g][0:64, sl],
                        start=(g == 0), stop=False, skip_group_check=True)
                for g in range(G):
                    nc.tensor.matmul(
                        lg32[:, sl], fq32g[g][:],
                        lt[:, g * N:(g + 1) * N][:, sl],
                        start=False, stop=(g == G - 1),
                        skip_group_check=True)
            # lgf = 10*tanh(x/sqrt(D)) + mask, in halves so the DVE
            # mask-add of half 0 overlaps the ACT tanh of half 1
            for half in range(2):
                sl = slice(half * 512, (half + 1) * 512)
                nc.scalar.activation(u32[:, sl], lg32[:, sl], AF_TANH,
                                     scale=float(1.0 / np.sqrt(D)))
                nc.vector.scalar_tensor_tensor(
                    lgf32[:, sl], u32[:, sl], TANH_CLIP, mbs32[:, sl],
                    op0=OP.mult, op1=OP.add)

            # ============ epilogue: batched flat log-softmax/argmax ======
            rs32 = S("rs32e", [32, 1])
            expf = S("expfe", [32, N])
            nc.scalar.activation(expf[:], lgf32[:], AF_EXP,
                                 accum_out=rs32[:])
            mx8 = S("mx8e", [32, 8])
            ix8 = S("ix8e", [32, 8], dt.uint32)
            nc.vector.max_with_indices(mx8[:], ix8[:], lgf32[:])
            idxf = S("idxfe", [32, 1])
            nc.vector.tensor_copy(idxf[:], ix8[:, 0:1])
            cand = S("cande", [32, 1])
            nc.vector.tensor_tensor(cand[:], c8k32, idxf[:], OP.subtract)

            rmt_ps = tp.tile([1, 32], f32, name="rmt_ps", tag="tp")
            nc.tensor.matmul(rmt_ps[:], mx8[:, 0:1], ident[0:32, 0:32],
                             is_transpose=True, start=True, stop=True)
            rmt = S("rmte", [1, 32])
            nc.vector.tensor_copy(rmt[:], rmt_ps[:])
            rst_ps = tp.tile([1, 32], f32, name="rst_ps", tag="tp")
            nc.tensor.matmul(rst_ps[:], rs32[:], ident[0:32, 0:32],
                             is_transpose=True, start=True, stop=True)
            rst = S("rste", [1, 32])
            nc.vector.tensor_copy(rst[:], rst_ps[:])
            cdt_ps = tp.tile([1, 32], f32, name="cdt_ps", tag="tp")
            nc.tensor.matmul(cdt_ps[:], cand[:], ident[0:32, 0:32],
                             is_transpose=True, start=True, stop=True)
            cdt = S("cdte", [1, 32])
            nc.vector.tensor_copy(cdt[:], cdt_ps[:])

            mt4 = S("mt4e", [1, G])
            nc.vector.tensor_reduce(mt4[:],
                                    rmt.rearrange("o (g v) -> o g v", g=G),
                                    AX.X, OP.max)
            s4 = S("s4e", [1, G])
            nc.vector.tensor_reduce(s4[:],
                                    rst.rearrange("o (g v) -> o g v", g=G),
                                    AX.X, OP.add)
            em4 = S("em4e", [1, G])
            nc.scalar.activation(em4[:], mt4[:], AF_EXP, scale=-1.0)
            s4p = S("s4pe", [1, G])
            nc.vector.tensor_tensor(s4p[:], s4[:], em4[:], OP.mult)
            lns4 = S("lns4e", [1, G])
            nc.scalar.activation(lns4[:], s4p[:], AF_LN)
            prob4 = S("prob4e", [1, G])
            nc.vector.reciprocal(prob4[:], s4p[:])
            mtb = (mt4.unsqueeze(2).broadcast_to([1, G, V]))
            eq = S("eqe", [1, 32])
            nc.vector.tensor_tensor(
                eq.rearrange("o (g v) -> o g v", g=G),
                rmt.rearrange("o (g v) -> o g v", g=G), mtb, OP.is_equal)
            cs = S("cse", [1, 32])
            nc.vector.tensor_tensor(cs[:], eq[:], cdt[:], OP.mult)
            cm4 = S("cm4e", [1, G])
            nc.vector.tensor_reduce(cm4[:],
                                    cs.rearrange("o (g v) -> o g v", g=G),
                                    AX.X, OP.max)
            res16 = S("res16e", [1, 4 * G])
            nc.vector.tensor_scalar(res16[:, 0:16:4], cm4[:], -1.0, 8192.0,
                                    OP.mult, OP.add)
            nc.vector.tensor_scalar_mul(res16[:, 1:16:4], lns4[:], -1.0)
            nc.vector.tensor_tensor(res16[:, 2:16:4], prob4[:], lns4[:],
                                    OP.mult)
            nc.vector.memset(res16[:, 3:16:4], 0.0)
            nc.sync.dma_start(
                res_out.ap().rearrange("a b -> (a b)").unsqueeze(0),
                res16[:])

    nc.compile()
    return nc


def _get_program():
    if "nc" not in _PROGRAM_CACHE:
        _PROGRAM_CACHE["nc"] = _build_program()
    return _PROGRAM_CACHE["nc"]


# --------------------------------------------------------------------------
# Host-side sharding / layout prep
# --------------------------------------------------------------------------

def _make_in_maps(inputs):
    gk = np.asarray(inputs["glimpse_K_static"], dtype=np.float32)
    gv = np.asarray(inputs["glimpse_V_static"], dtype=np.float32)
    lk = np.asarray(inputs["logit_K_static"], dtype=np.float32)
    ndf = np.asarray(inputs["node_dynamic_features"], dtype=np.float32)
    vdf = np.asarray(inputs["vehicle_dynamic_features"], dtype=np.float32)
    fc = np.asarray(inputs["fixed_context"], dtype=np.float32)
    msk = np.asarray(inputs["feasibility_mask"])
    w_cs = np.asarray(inputs["W_cs"], dtype=np.float32)
    w_ns = np.asarray(inputs["W_ns"], dtype=np.float32)
    w_out = np.asarray(inputs["W_out"], dtype=np.float32)

    in_maps = []
    for c in range(NCORES):
        bs = slice(c * G, (c + 1) * G)
        kt = np.ascontiguousarray(
            gk[:, bs].transpose(1, 0, 3, 2).reshape(G, 128, N))
        lt = np.ascontiguousarray(
            lk[bs].transpose(0, 2, 1).transpose(1, 0, 2).reshape(128, G * N))
        vn = gv[:, bs].transpose(1, 2, 0, 3).reshape(G, N, 128)
        nd = ndf[bs]                                   # [G, V, N, F]
        ndftm = np.zeros((G, 72, N), dtype=np.float32)
        ndftm[:, 0:64, :] = nd.transpose(0, 1, 3, 2).reshape(G, 64, N)
        mbx = (msk[bs].astype(np.float32) - 1.0) * MASK_BIG   # [G, V, N]
        ndftm[:, 64:72, :] = mbx
        mbs32 = np.ascontiguousarray(mbx.reshape(32, N))
        ndfn = (nd.reshape(NPAIR, 2, V, N, F_ND)
                .transpose(0, 3, 1, 2, 4).reshape(NPAIR, N, 128))
        rhsha = np.empty((NPAIR, 128, 24, 128), dtype=np.float32)
        for p in range(NPAIR):
            for g2 in range(2):
                rhsha[p, :, g2 * 8:(g2 + 1) * 8, :] = (
                    vn[2 * p + g2].reshape(8, 128, 128).transpose(1, 0, 2))
            rhsha[p, :, 16:24, :] = (
                ndfn[p].reshape(8, 128, 128).transpose(1, 0, 2))
        rhsha = np.ascontiguousarray(rhsha.reshape(NPAIR, 128, 3 * N))
        wpack = np.zeros((128, 564), dtype=np.float32)
        wpack[:, 0:128] = w_cs[:D]
        wpack[:, 128:256] = w_out
        wpack[0:F_ND, 256:384] = w_ns[:, 0:D]
        wpack[0:F_V, 384:512] = w_cs[D:]
        wpack[:, 512:516] = fc[bs].T
        wpack[0:F_V, 516:548] = vdf[bs].transpose(2, 0, 1).reshape(F_V, 32)
        wpack[:, 548:556] = w_ns[:, D:2 * D].T
        wpack[:, 556:564] = w_ns[:, 2 * D:3 * D].T
        in_maps.append({
            "kt_in": kt,
            "lt_in": lt,
            "rhsha_in": rhsha,
            "ndftm_in": ndftm,
            "mbs32_in": mbs32,
            "wpack_in": wpack,
        })
    return in_maps


def _postprocess(res_list):
    out = np.concatenate(res_list, axis=0)              # [B, 4]
    a = out[:, 0]
    lp = out[:, 1].astype(np.float32)
    ent = out[:, 2].astype(np.float32)
    sel_vec = (a.astype(np.float32) / np.float32(N)).astype(np.float32)
    sel_node = (np.round(a).astype(np.int64) % N).astype(np.int32)
    return sel_vec, sel_node, lp, ent


LAST_RESULTS = None
ENABLE_LDW_OPT = True
_LDW_PATCHED = False


def _patch_ldw_opt():
    """Flip walrus --enable-ldw-opt (elides redundant PE weight loads)."""
    global _LDW_PATCHED
    if _LDW_PATCHED or not ENABLE_LDW_OPT:
        return
    import concourse.bass_utils as bu
    orig = bu.run_command

    def patched(argv, **kw):
        argv = ["--enable-ldw-opt=true" if a == "--enable-ldw-opt=false"
                else a for a in argv]
        return orig(argv, **kw)

    bu.run_command = patched
    _LDW_PATCHED = True


def _run(inputs, trace=False):
    global LAST_RESULTS
    _patch_ldw_opt()
    from concourse.bass_utils import run_bass_kernel_spmd
    nc = _get_program()
    in_maps = _make_in_maps(inputs)
    res = run_bass_kernel_spmd(nc, in_maps, list(range(NCORES)), trace=trace)
    LAST_RESULTS = res
    return _postprocess([res.results[c]["res_out"] for c in range(NCORES)])


def kernel(**inputs):
    return _run(inputs, trace=False)
